# revision 21
# baseline (speedup 1.0000x reference)
"""Trainium2 Bass kernel for the 4-layer sum/product circuit (segment_reduce).

Strategy: shard batch (4096) across 8 cores (512 each), zero communication.
Each layer is a row-gather + k-leg reduce. Gathers run as SWDGE dma_gather
(GPSIMD desc-gen on 4 rotating SWDGE queues = 4 Q7 cpu pairs; transfers on
the 16 DMA engines) from f16 HBM tables whose rows hold one node's 512 batch
values; layer outputs are written back as the next table. Legs are
host-reordered to t = (s%128) + 128*((s//128)*k + j) so a gathered
super-chunk lands with output slot s's k legs adjacent on partition s%128 —
the reduce is (k-1) contiguous f16 DVE adds and the result lands in table
row order (no transposes; leg values are host-remapped to storage rows).
ACT applies Exp (product->sum handoff) / Ln (sum layers) on the way to the
table write.

Layer outputs are additionally sorted by their max source row, and each
gather's in_ap is narrowed to the row prefix it actually reads, quantized to
super-chunk writes: the tile scheduler then overlaps a layer's early gathers
with the previous layer's tail writes instead of draining at each boundary.
"""

import math
import numpy as np
from contextlib import ExitStack

import concourse.bacc as bacc
import concourse.tile as tile
from concourse import bass, mybir
from concourse import library_config
from concourse.bass_utils import run_bass_kernel_spmd

N_CORES = 8
B = 4096
BPC = B // N_CORES          # 512 batch per core
E = BPC                     # gather row length (elements)

N_XENC = 2050
LAYERS = [
    # (name, n_out, k, QC per super-chunk). Each super-chunk is 128*k*QC =
    # 4096 legs, gathered by 4 dma_gather calls of 1024 idx each (>= ~2048
    # idx per gather overflows the SWDGE descriptor ring) on rotating SWDGE
    # queues, then reduced/activated/written as one unit.
    ("l1", 8192, 4, 8),
    ("l2", 4096, 8, 4),
    ("l3", 8192, 4, 8),
    ("l4", 2048, 8, 4),
]
GSPLIT = 4                  # dma_gather calls per super-chunk


def _wrap_idx(flat_idx: np.ndarray) -> np.ndarray:
    """[Q] int -> [128, Q//16] int16 wrapped per 16 partitions, replicated x8."""
    q = flat_idx.shape[0]
    assert q % 16 == 0
    w = flat_idx.reshape(q // 16, 16).T.astype(np.int16)
    return np.tile(w, (8, 1))


def _log1mexp(x):
    # match reference (Maechler 2012) in f32
    x = x.astype(np.float32)
    with np.errstate(divide="ignore", invalid="ignore"):
        a = np.log(-np.expm1(x)).astype(np.float32)
        b = np.log1p(-np.exp(x)).astype(np.float32)
    return np.where(x > -math.log(2.0), a, b).astype(np.float32)


def _plan(idx_list):
    """Host planning: per layer, sort outputs by max source row (ascending),
    remap leg values to previous-layer storage rows, flatten legs in
    t = (s%128) + 128*((s//128)*k + j) order, and compute per-sub-gather
    source-row bounds quantized to previous-layer super-chunk writes.

    Returns (wrapped leg arrays per layer, rmax chunks per layer, final perm).
    """
    legs_out, deps_out = [], []
    prev_rank = None        # original id -> storage row of previous table
    prev_wchunk = None      # rows per previous-layer super-chunk write
    for li, (idx, (_, n_out, k, qc)) in enumerate(zip(idx_list, LAYERS)):
        v = idx.astype(np.int64)
        if prev_rank is not None:
            v = prev_rank[v]
        if li == 0:
            order = np.arange(n_out)         # xenc fully available; no sort
        else:
            order = np.argsort(v.max(axis=1), kind="stable")
        v = v[order]
        rank = np.empty(n_out, np.int64)
        rank[order] = np.arange(n_out)
        # legs in slot order: [Q, 128, k] -> [Q, k, 128]
        flat = v.reshape(n_out // 128, 128, k).transpose(0, 2, 1).reshape(-1)
        legs_out.append(_wrap_idx(flat))
        # per-sub-gather row bound, quantized up to prev write chunks
        sub = 128 * k * qc // GSPLIT
        if li == 0:
            deps_out.append(None)
        else:
            mx = flat.reshape(-1, sub).max(axis=1) + 1
            deps_out.append(np.minimum(
                np.ceil(mx / prev_wchunk).astype(np.int64) * prev_wchunk,
                LAYERS[li - 1][1]))
        prev_rank = rank
        prev_wchunk = 128 * qc
    return legs_out, deps_out, prev_rank


def _build(nc, deps):
    f16 = mybir.dt.float16
    f32 = mybir.dt.float32
    i16 = mybir.dt.int16
    add = mybir.AluOpType.add
    EXP = mybir.ActivationFunctionType.Exp
    LN = mybir.ActivationFunctionType.Ln

    xenc_d = nc.dram_tensor("xenc", [N_XENC, E], f16, kind="ExternalInput")
    idx_d, tab_d = {}, {}
    for name, n_out, k, _ in LAYERS:
        idx_d[name] = nc.dram_tensor(f"{name}idx", [128, n_out * k // 16], i16,
                                     kind="ExternalInput")
    for name, n_out, _, _ in LAYERS[:-1]:
        tab_d[name] = nc.dram_tensor(f"{name}tab", [n_out, E], f16,
                                     kind="Internal")
    out_d = nc.dram_tensor("out_t", [LAYERS[-1][1], E], f32,
                           kind="ExternalOutput")

    with tile.TileContext(nc) as tc, ExitStack() as ctx:
        nc.gpsimd.load_library(library_config.mlp)
        idxp = ctx.enter_context(tc.tile_pool(name="idxp", bufs=1))
        gpool = ctx.enter_context(tc.tile_pool(name="gpool", bufs=4))
        apool = ctx.enter_context(tc.tile_pool(name="apool", bufs=3))
        spool = ctx.enter_context(tc.tile_pool(name="spool", bufs=3))
        opool = ctx.enter_context(tc.tile_pool(name="opool", bufs=2))

        idx_t = {}
        for name, n_out, k, _ in LAYERS:
            t = idxp.tile([128, n_out * k // 16], i16, tag=f"idx{name}")
            nc.sync.dma_start(t[:], idx_d[name][:])
            idx_t[name] = t

        srcs = {"l1": xenc_d, "l2": tab_d["l1"], "l3": tab_d["l2"],
                "l4": tab_d["l3"]}
        acts = {"l1": EXP, "l2": LN, "l3": EXP, "l4": LN}

        gq = 0
        for li, (name, n_out, k, qc) in enumerate(LAYERS):
            src = srcs[name]
            last = li == len(LAYERS) - 1
            dst = (out_d if last else tab_d[name])[:].rearrange(
                "(q p) e -> p q e", p=128)
            odt = f32 if last else f16
            ch = 128 * k * qc          # legs per super-chunk
            sub = ch // GSPLIT         # legs per dma_gather call
            slots = sub // 128         # gather-buf columns per call
            n_ch = n_out // 128 // qc
            def emit_reduce(g, c0, c1, q0, nq):
                """Reduce gather-buf columns [c0:c1) -> outputs [q0, q0+nq)."""
                ga = g[:, c0:c1, :].rearrange("p (q k) e -> p q k e", k=k)
                acc_t = apool.tile([128, 8, E], f16, tag="acc")
                acc = acc_t[:, :nq, :]
                nc.vector.tensor_tensor(acc, ga[:, :, 0, :], ga[:, :, 1, :],
                                        add)
                for j in range(2, k):
                    nc.vector.tensor_tensor(acc, acc, ga[:, :, j, :], add)
                if last:
                    st_t = opool.tile([128, qc, E], f32, tag="out")
                else:
                    st_t = spool.tile([128, 8, E], f16, tag="st")
                st = st_t[:, :nq, :]
                nc.scalar.activation(st, acc, acts[name])
                nc.sync.dma_start(dst[:, q0:q0 + nq, :], st)

            for ci in range(n_ch):
                # the kernel's first and last super-chunks are processed per
                # sub-gather: faster pipeline fill and a shorter final drain
                split = (last and ci == n_ch - 1) or (li == 0 and ci == 0)
                g = gpool.tile([128, qc * k, E], f16, tag="g")
                for s in range(GSPLIT):
                    gi = ci * GSPLIT + s
                    rmax = N_XENC if deps[li] is None else int(deps[li][gi])
                    i0 = (ci * ch + s * sub) // 16
                    nc.gpsimd.dma_gather(
                        g[:, s * slots:(s + 1) * slots, :], src[:rmax, :],
                        idx_t[name][:, i0:i0 + sub // 16],
                        sub, sub, E, queue_num=gq % 4)
                    gq += 1
                    if split:
                        sq = max(qc // GSPLIT, 1)
                        emit_reduce(g, s * slots, (s + 1) * slots,
                                    ci * qc + s * sq, sq)
                if not split:
                    emit_reduce(g, 0, qc * k, ci * qc, qc)
    nc.compile()
    return nc


_CACHED_NC = None
_CACHED_KEY = None
_LAST_IN_MAPS = None


def kernel(pos, idx0, idx1, idx2, idx3):
    global _CACHED_NC, _CACHED_KEY, _LAST_IN_MAPS
    pos = np.asarray(pos, dtype=np.float32)

    # host-side input encoding: x_enc [2050, 4096]
    neg = _log1mexp(pos)
    n, b = pos.shape
    xenc = np.zeros((2 * n + 2, b), np.float32)
    xenc[1] = 0.0
    xenc[2::2] = pos
    xenc[3::2] = neg
    # row 0 is -inf in the reference but never gathered (idx0 >= 1); keep 0.
    xenc16 = xenc.astype(np.float16)

    legs, deps, out_perm = _plan(
        [np.asarray(a) for a in (idx0, idx1, idx2, idx3)])
    idx_maps = {f"{name}idx": w for (name, _, _, _), w in zip(LAYERS, legs)}

    key = tuple(tuple(d) for d in deps[1:] if d is not None)
    if _CACHED_NC is None or _CACHED_KEY != key:
        _CACHED_NC = _build(bacc.Bacc("TRN2", target_bir_lowering=False,
                                      debug=False, num_swdge_queues=4), deps)
        _CACHED_KEY = key
    nc = _CACHED_NC

    in_maps = []
    for c in range(N_CORES):
        in_maps.append({
            "xenc": np.ascontiguousarray(xenc16[:, c * BPC:(c + 1) * BPC]),
            **idx_maps,
        })
    _LAST_IN_MAPS = in_maps
    res = run_bass_kernel_spmd(nc, in_maps, list(range(N_CORES)))

    n_out = LAYERS[-1][1]
    out = np.empty((n_out, B), np.float32)
    for c in range(N_CORES):
        ot = res.results[c]["out_t"].reshape(n_out, BPC)
        # slot s holds original output o with out_perm[o] = s
        out[:, c * BPC:(c + 1) * BPC] = ot[out_perm]
    return out


# revision 22
# speedup vs baseline: 1.0563x; 1.0563x over previous
"""Trainium2 Bass kernel for the 4-layer sum/product circuit (segment_reduce).

Strategy: shard batch (4096) across 8 cores (512 each), zero communication.
Each layer is a row-gather + k-leg reduce. Gathers run as SWDGE dma_gather
(GPSIMD desc-gen on 4 rotating SWDGE queues = 4 Q7 cpu pairs; transfers on
the 16 DMA engines) from f16 HBM tables whose rows hold one node's 512 batch
values; layer outputs are written back as the next table. Legs are
host-reordered to t = (s%128) + 128*((s//128)*k + j) so a gathered
super-chunk lands with output slot s's k legs adjacent on partition s%128 —
the reduce is (k-1) contiguous f16 DVE adds and the result lands in table
row order (no transposes; leg values are host-remapped to storage rows).
ACT applies Exp (product->sum handoff) / Ln (sum layers) on the way to the
table write.

Layer outputs are additionally sorted by their max source row, and each
gather's in_ap is narrowed to the row prefix it actually reads, quantized to
super-chunk writes: the tile scheduler then overlaps a layer's early gathers
with the previous layer's tail writes instead of draining at each boundary.
"""

import math
import numpy as np
from contextlib import ExitStack

import concourse.bacc as bacc
import concourse.tile as tile
from concourse import bass, mybir
from concourse import library_config
from concourse.bass_utils import run_bass_kernel_spmd

N_CORES = 8
B = 4096
BPC = B // N_CORES          # 512 batch per core
E = BPC                     # gather row length (elements)

N_XENC = 2050
LAYERS = [
    # (name, n_out, k, QC per super-chunk). Each super-chunk is 128*k*QC =
    # 4096 legs, gathered by 4 dma_gather calls of 1024 idx each (>= ~2048
    # idx per gather overflows the SWDGE descriptor ring) on rotating SWDGE
    # queues, then reduced/activated/written as one unit.
    ("l1", 8192, 4, 8),
    ("l2", 4096, 8, 4),
    ("l3", 8192, 4, 8),
    ("l4", 2048, 8, 4),
]
GSPLIT = 4                  # dma_gather calls per super-chunk


def _wrap_idx(flat_idx: np.ndarray) -> np.ndarray:
    """[Q] int -> [128, Q//16] int16 wrapped per 16 partitions, replicated x8."""
    q = flat_idx.shape[0]
    assert q % 16 == 0
    w = flat_idx.reshape(q // 16, 16).T.astype(np.int16)
    return np.tile(w, (8, 1))


def _log1mexp(x):
    # match reference (Maechler 2012) in f32
    x = x.astype(np.float32)
    with np.errstate(divide="ignore", invalid="ignore"):
        a = np.log(-np.expm1(x)).astype(np.float32)
        b = np.log1p(-np.exp(x)).astype(np.float32)
    return np.where(x > -math.log(2.0), a, b).astype(np.float32)


def _plan(idx_list):
    """Host planning: per layer, sort outputs by max source row (ascending),
    remap leg values to previous-layer storage rows, flatten legs in
    t = (s%128) + 128*((s//128)*k + j) order, and compute per-sub-gather
    source-row bounds quantized to previous-layer super-chunk writes.

    Returns (wrapped leg arrays per layer, rmax chunks per layer, final perm).
    """
    legs_out, deps_out = [], []
    prev_rank = None        # original id -> storage row of previous table
    prev_wchunk = None      # rows per previous-layer super-chunk write
    for li, (idx, (_, n_out, k, qc)) in enumerate(zip(idx_list, LAYERS)):
        v = idx.astype(np.int64)
        if prev_rank is not None:
            v = prev_rank[v]
        if li == 0:
            order = np.arange(n_out)         # xenc fully available; no sort
        else:
            order = np.argsort(v.max(axis=1), kind="stable")
        v = v[order]
        rank = np.empty(n_out, np.int64)
        rank[order] = np.arange(n_out)
        # legs in slot order: [Q, 128, k] -> [Q, k, 128]
        flat = v.reshape(n_out // 128, 128, k).transpose(0, 2, 1).reshape(-1)
        legs_out.append(_wrap_idx(flat))
        # per-sub-gather row bound, quantized up to prev write chunks
        sub = 128 * k * qc // GSPLIT
        if li == 0:
            deps_out.append(None)
        else:
            mx = flat.reshape(-1, sub).max(axis=1) + 1
            deps_out.append(np.minimum(
                np.ceil(mx / prev_wchunk).astype(np.int64) * prev_wchunk,
                LAYERS[li - 1][1]))
        prev_rank = rank
        prev_wchunk = 128 * qc
    return legs_out, deps_out, prev_rank


def _build(nc, deps):
    f16 = mybir.dt.float16
    f32 = mybir.dt.float32
    i16 = mybir.dt.int16
    add = mybir.AluOpType.add
    EXP = mybir.ActivationFunctionType.Exp
    LN = mybir.ActivationFunctionType.Ln

    xenc_d = nc.dram_tensor("xenc", [N_XENC, E], f16, kind="ExternalInput")
    idx_d, tab_d = {}, {}
    for name, n_out, k, _ in LAYERS:
        idx_d[name] = nc.dram_tensor(f"{name}idx", [128, n_out * k // 16], i16,
                                     kind="ExternalInput")
    for name, n_out, _, _ in LAYERS[:-1]:
        tab_d[name] = nc.dram_tensor(f"{name}tab", [n_out, E], f16,
                                     kind="Internal")
    out_d = nc.dram_tensor("out_t", [LAYERS[-1][1], E], f32,
                           kind="ExternalOutput")

    with tile.TileContext(nc) as tc, ExitStack() as ctx:
        nc.gpsimd.load_library(library_config.mlp)
        idxp = ctx.enter_context(tc.tile_pool(name="idxp", bufs=1))
        gpool = ctx.enter_context(tc.tile_pool(name="gpool", bufs=3))
        apool = ctx.enter_context(tc.tile_pool(name="apool", bufs=4))
        spool = ctx.enter_context(tc.tile_pool(name="spool", bufs=3))
        opool = ctx.enter_context(tc.tile_pool(name="opool", bufs=2))

        idx_t = {}
        for name, n_out, k, _ in LAYERS:
            t = idxp.tile([128, n_out * k // 16], i16, tag=f"idx{name}")
            nc.sync.dma_start(t[:], idx_d[name][:])
            idx_t[name] = t

        srcs = {"l1": xenc_d, "l2": tab_d["l1"], "l3": tab_d["l2"],
                "l4": tab_d["l3"]}
        acts = {"l1": EXP, "l2": LN, "l3": EXP, "l4": LN}

        gq = 0
        for li, (name, n_out, k, qc) in enumerate(LAYERS):
            src = srcs[name]
            last = li == len(LAYERS) - 1
            dst = (out_d if last else tab_d[name])[:].rearrange(
                "(q p) e -> p q e", p=128)
            odt = f32 if last else f16
            ch = 128 * k * qc          # legs per super-chunk
            sub = ch // GSPLIT         # legs per dma_gather call
            slots = sub // 128         # gather-buf columns per call
            n_ch = n_out // 128 // qc
            def emit_reduce(g, c0, c1, q0, nq):
                """Reduce gather-buf columns [c0:c1) -> outputs [q0, q0+nq)."""
                ga = g[:, c0:c1, :].rearrange("p (q k) e -> p q k e", k=k)
                acc_t = apool.tile([128, 8, E], f16, tag="acc")
                acc = acc_t[:, :nq, :]
                nc.vector.tensor_tensor(acc, ga[:, :, 0, :], ga[:, :, 1, :],
                                        add)
                for j in range(2, k):
                    nc.vector.tensor_tensor(acc, acc, ga[:, :, j, :], add)
                if last:
                    st_t = opool.tile([128, qc, E], f32, tag="out")
                else:
                    st_t = spool.tile([128, 8, E], f16, tag="st")
                st = st_t[:, :nq, :]
                nc.scalar.activation(st, acc, acts[name])
                nc.sync.dma_start(dst[:, q0:q0 + nq, :], st)

            for ci in range(n_ch):
                # the kernel's first and last super-chunks are processed per
                # sub-gather: faster pipeline fill and a shorter final drain
                split = (last and ci == n_ch - 1) or (li == 0 and ci == 0)
                g = gpool.tile([128, qc * k, E], f16, tag="g")
                for s in range(GSPLIT):
                    gi = ci * GSPLIT + s
                    rmax = N_XENC if deps[li] is None else int(deps[li][gi])
                    i0 = (ci * ch + s * sub) // 16
                    nc.gpsimd.dma_gather(
                        g[:, s * slots:(s + 1) * slots, :], src[:rmax, :],
                        idx_t[name][:, i0:i0 + sub // 16],
                        sub, sub, E, queue_num=gq % 4)
                    gq += 1
                    if split:
                        sq = max(qc // GSPLIT, 1)
                        emit_reduce(g, s * slots, (s + 1) * slots,
                                    ci * qc + s * sq, sq)
                if not split:
                    emit_reduce(g, 0, qc * k, ci * qc, qc)
    nc.compile()
    return nc


_CACHED_NC = None
_CACHED_KEY = None
_LAST_IN_MAPS = None


def kernel(pos, idx0, idx1, idx2, idx3):
    global _CACHED_NC, _CACHED_KEY, _LAST_IN_MAPS
    pos = np.asarray(pos, dtype=np.float32)

    # host-side input encoding: x_enc [2050, 4096]
    neg = _log1mexp(pos)
    n, b = pos.shape
    xenc = np.zeros((2 * n + 2, b), np.float32)
    xenc[1] = 0.0
    xenc[2::2] = pos
    xenc[3::2] = neg
    # row 0 is -inf in the reference but never gathered (idx0 >= 1); keep 0.
    xenc16 = xenc.astype(np.float16)

    legs, deps, out_perm = _plan(
        [np.asarray(a) for a in (idx0, idx1, idx2, idx3)])
    idx_maps = {f"{name}idx": w for (name, _, _, _), w in zip(LAYERS, legs)}

    key = tuple(tuple(d) for d in deps[1:] if d is not None)
    if _CACHED_NC is None or _CACHED_KEY != key:
        _CACHED_NC = _build(bacc.Bacc("TRN2", target_bir_lowering=False,
                                      debug=False, num_swdge_queues=4), deps)
        _CACHED_KEY = key
    nc = _CACHED_NC

    in_maps = []
    for c in range(N_CORES):
        in_maps.append({
            "xenc": np.ascontiguousarray(xenc16[:, c * BPC:(c + 1) * BPC]),
            **idx_maps,
        })
    _LAST_IN_MAPS = in_maps
    res = run_bass_kernel_spmd(nc, in_maps, list(range(N_CORES)))

    n_out = LAYERS[-1][1]
    out = np.empty((n_out, B), np.float32)
    for c in range(N_CORES):
        ot = res.results[c]["out_t"].reshape(n_out, BPC)
        # slot s holds original output o with out_perm[o] = s
        out[:, c * BPC:(c + 1) * BPC] = ot[out_perm]
    return out
